# revision 2
# baseline (speedup 1.0000x reference)
"""
CRFTagger NLL loss on 8 Trainium2 NeuronCores (Bass/Tile).

Strategy
--------
Data-parallel over batch: each of the 8 cores runs the CRF forward algorithm
for 16 of the 128 sequences.  The log-semiring scan runs in the *exp domain*
with a constant per-step shift s (s = log Perron-eigenvalue of exp(trans)
+ 0.5, computed on host from the transitions input), so one scan step is
just one PE matmul + one DVE elementwise multiply:

    P_{t+1} = (E^T @ P_t) * exp(feat_t)        E = exp(trans - s)  [C,C]

No per-step logsumexp / max / renormalization: with the Perron shift the
magnitudes drift only a few e-folds over the whole scan (validated:
|log P| < 10); states/weights are bf16 (fp32 exponent range, overflow-proof).

The scan is a latency-bound serial chain (~435ns per matmul+mul round trip),
so the kernel halves the chain length with a *bidirectional* split: a forward
recursion over the first half of time and an independent backward recursion
over the second half run concurrently, interleaved on the PE and DVE engines.
For variable-length sequences the backward pass is time-ALIGNED on host: the
feature stream of sequence b is reversed and shifted so every sequence "ends"
at the same device iteration, making the backward init a single shared
one-hot STOP seed.  Both chains run exactly NS=256 steps and meet with NO
feature overlap; the host closes the gap with one tiny E-matvec per
sequence:

    logZ_b = log( Pf_t^T  E  Xb_m ) + (len_b + 1) * s ,
    t = min(len_b, 256),  m = len_b - t

Both chains store their full state history in SBUF and ship it out
(overlapped with compute).  The history is laid out so the *tail* slots of
both chains (248..256) sit in one contiguous column range, letting the final
unoverlapped transfer be a single small DMA.  Regular ship points are chosen
off the feature-chunk boundaries (and staggered F vs B) so the ship DMAs
never collide with the chunk handoff on the serial chain.

The gold-path score (pure gathers over tags, O(B*T) with zero reuse) is
evaluated on host during unsharding.
"""

import sys

import ml_dtypes
import numpy as np

sys.path.insert(0, "/opt/trn_rl_repo")

import concourse.bacc as bacc  # noqa: E402
import concourse.mybir as mybir  # noqa: E402
from concourse import tile  # noqa: E402
from concourse.bass_utils import run_bass_kernel_spmd  # noqa: E402
from concourse.tile_rust import add_dep_helper  # noqa: E402

B, T, C = 128, 512, 128
N_CORES = 8
BL = B // N_CORES   # 16 sequences per core
NS = T // 2         # 256 steps per chain (both directions)

# state-history slot -> column layout: slots 0..247 of each chain live in
# two contiguous "body" regions, slots 248..NS (the tail, still in flight
# when the scan ends) of BOTH chains share one contiguous range so the
# final unoverlapped ship is a single DMA.
TAIL0 = 248                     # first tail slot
NTAIL = NS - TAIL0 + 1          # 9 tail slots per chain
HSLOTS = 2 * TAIL0 + 2 * NTAIL  # 514 total slots


def _col_f(s):  # forward-chain slot -> column group
    return s if s < TAIL0 else 2 * TAIL0 + (s - TAIL0)


def _col_b(s):  # backward-chain slot -> column group
    return TAIL0 + s if s < TAIL0 else 2 * TAIL0 + NTAIL + (s - TAIL0)


# feature chunk spans (steps); first chunk tiny so the scan starts after a
# small DMA+exp; F and B switch points staggered so the two chains never
# hit a chunk handoff on the same period
FW_SPANS = [(0, 16), (16, 80), (80, 144), (144, 208), (208, 256)]
BW_SPANS = [(0, 16), (16, 48), (48, 112), (112, 176), (176, 240), (240, 256)]
# ship points (fire after the TT that writes slot k+1 == point); all <= 247
# so regular ships stay inside the contiguous body regions, and none equals
# a chunk-switch step of either chain
FW_SHIPS = (64, 128, 192, 247)
BW_SHIPS = (32, 96, 160, 224, 247)

_NC = None
LAST_RESULT = None  # BassKernelResults of the most recent run (for profiling)


def _build_nc():
    nc = bacc.Bacc("TRN2", target_bir_lowering=False, debug=False)
    fp32 = mybir.dt.float32
    fp16 = mybir.dt.bfloat16
    ffw_h = nc.dram_tensor("ffw", [C, NS, BL], fp32, kind="ExternalInput")
    fbw_h = nc.dram_tensor("fbw", [C, NS, BL], fp32, kind="ExternalInput")
    # one constant block = one DMA: [E | E^T | seedF | seedB]
    konst_h = nc.dram_tensor(
        "konst", [C, 2 * C + 2 * BL], fp16, kind="ExternalInput"
    )
    hist_h = nc.dram_tensor("hist", [C, HSLOTS * BL], fp16, kind="ExternalOutput")

    with tile.TileContext(nc) as tc:
        with (
            tc.tile_pool(name="consts", bufs=1) as consts,
            tc.tile_pool(name="ffw", bufs=len(FW_SPANS)) as ffwp,
            tc.tile_pool(name="fbw", bufs=len(BW_SPANS)) as fbwp,
            tc.tile_pool(name="hist", bufs=1) as hist,
            tc.tile_pool(name="mpsF", bufs=2, space="PSUM") as mpsF,
            tc.tile_pool(name="mpsB", bufs=2, space="PSUM") as mpsB,
        ):
            konst = consts.tile([C, 2 * C + 2 * BL], fp16)
            nc.sync.dma_start(out=konst[:], in_=konst_h[:])
            emat = konst[:, 0:C]
            ematT = konst[:, C : 2 * C]
            seedF = konst[:, 2 * C : 2 * C + BL]
            seedB = konst[:, 2 * C + BL : 2 * C + 2 * BL]

            # both chains' state histories in one tile (slot->column maps
            # above); slot 0 (the seed) lives in the konst tile instead —
            # the host never reads slot 0 of the shipped history.
            HIST = hist.tile([C, HSLOTS * BL], fp16)

            # stream feats in chunks, exponentiating in place
            def load_feats(pool, dram, lo, hi):
                f = pool.tile([C, (hi - lo) * BL], fp32)
                nc.sync.dma_start(
                    out=f[:],
                    in_=dram[:, lo:hi, :].rearrange("c t b -> c (t b)"),
                )
                nc.scalar.activation(
                    f[:], f[:], mybir.ActivationFunctionType.Exp
                )
                return f

            def slot_map(spans):
                m = {}
                for i, (lo, hi) in enumerate(spans):
                    for k in range(lo, hi):
                        m[k] = (i, k - lo)
                return m

            # first chunk of each chain is DMA'd/exp'd first so the scan can
            # start while the remaining chunks stream in; interleave F/B
            # chunk loads so neither chain's next chunk gets queued behind
            # all of the other chain's DMAs
            ffw, fbw = [], []
            for i in range(max(len(FW_SPANS), len(BW_SPANS))):
                if i < len(FW_SPANS):
                    ffw.append(load_feats(ffwp, ffw_h, *FW_SPANS[i]))
                if i < len(BW_SPANS):
                    fbw.append(load_feats(fbwp, fbw_h, *BW_SPANS[i]))
            fw_slot = slot_map(FW_SPANS)
            bw_slot = slot_map(BW_SPANS)

            def step(k, psum_pool, wmat, colmap, ftiles, fslot, seed,
                     ships, shipped, phase_after=None):
                m = psum_pool.tile([C, BL], mybir.dt.float32)
                if k == 0:
                    rhs = seed
                else:
                    c0 = colmap(k) * BL
                    rhs = HIST[:, c0 : c0 + BL]
                mm = nc.tensor.matmul(
                    m[:], wmat, rhs, start=True, stop=True,
                )
                if phase_after is not None:
                    # pure scheduling edge: pins this chain's phase a fixed
                    # lag behind the other chain so the two never collapse
                    # into the in-phase (serialized, 2x slower) mode
                    add_dep_helper(
                        mm.ins, phase_after.ins, sync=True,
                        reason="cross-chain phase pin",
                    )
                i, j = fslot[k]
                f = ftiles[i]
                co = colmap(k + 1) * BL
                tt = nc.vector.tensor_mul(
                    HIST[:, co : co + BL],
                    f[:, j * BL : (j + 1) * BL],
                    m[:],
                )
                # ship finished body slots while the scan keeps running
                if k + 1 in ships:
                    lo = colmap(shipped["s"]) * BL
                    hi = (colmap(k + 1) + 1) * BL
                    nc.sync.dma_start(
                        out=hist_h[:, lo:hi], in_=HIST[:, lo:hi]
                    )
                    shipped["s"] = k + 2
                return tt

            # Stagger the backward chain one step behind the forward chain in
            # each engine's (in-order) instruction stream, and pin its phase
            # with an explicit cross-chain edge, so B's ops always slot into
            # the idle gaps of F's latency-bound period instead of collapsing
            # into the in-phase (serialized, 2x slower) mode.
            prev_ttF = None
            shipF, shipB = {"s": 1}, {"s": 1}
            for k in range(NS):
                ttF = step(k, mpsF, emat, _col_f, ffw, fw_slot, seedF,
                           FW_SHIPS, shipF)
                if k >= 1:
                    step(k - 1, mpsB, ematT, _col_b, fbw, bw_slot, seedB,
                         BW_SHIPS, shipB, phase_after=prev_ttF)
                prev_ttF = ttF
            step(NS - 1, mpsB, ematT, _col_b, fbw, bw_slot, seedB,
                 BW_SHIPS, shipB, phase_after=prev_ttF)

            # single merged tail ship: slots 248..256 of BOTH chains are one
            # contiguous column range by construction
            lo = 2 * TAIL0 * BL
            nc.sync.dma_start(
                out=hist_h[:, lo:], in_=HIST[:, lo:]
            )
    nc.compile()
    return nc


def _get_nc():
    global _NC
    if _NC is None:
        _NC = _build_nc()
    return _NC


def _shift_constant(transitions: np.ndarray) -> float:
    """log(Perron eigenvalue of exp(trans)) + E[e^feat] growth correction."""
    tm = transitions.astype(np.float64)
    mx = tm.max()
    Et = np.exp(tm - mx)
    v = np.ones(C) / C
    r = 1.0
    for _ in range(200):
        w = Et.T @ v
        r = np.linalg.norm(w)
        v = w / r
    return float(np.log(r) + mx + 0.5)


def kernel(feats, mask, tags, transitions):
    global LAST_RESULT
    feats = np.asarray(feats, dtype=np.float32)
    mask = np.asarray(mask, dtype=np.int32)
    tags = np.asarray(tags, dtype=np.int32)
    transitions = np.asarray(transitions, dtype=np.float32)

    s = _shift_constant(transitions)
    with np.errstate(under="ignore"):
        emat = np.exp(
            (transitions.astype(np.float64) - s).astype(np.float32)
        ).astype(ml_dtypes.bfloat16)

    konst = np.zeros((C, 2 * C + 2 * BL), dtype=ml_dtypes.bfloat16)
    konst[:, :C] = emat
    konst[:, C : 2 * C] = emat.T
    konst[C - 2, 2 * C : 2 * C + BL] = 1.0        # forward seed: START one-hot
    konst[C - 1, 2 * C + BL : 2 * C + 2 * BL] = 1.0  # backward seed: STOP

    lengths = mask.sum(1)  # [B]

    # forward feats: [B,T,C] -> [C, NS, B] slices per core
    featsT = np.ascontiguousarray(feats[:, :NS, :].transpose(2, 1, 0))
    # backward aligned feats: iteration m of sequence b consumes
    # feats[b, len_b-1-m, :]; entries past the sequence start are 0 (exp -> 1)
    fbw_all = np.zeros((B, NS, C), dtype=np.float32)
    for b in range(B):
        L = int(lengths[b])
        n = min(L, NS)
        fbw_all[b, :n] = feats[b, L - n : L][::-1]
    fbwT = np.ascontiguousarray(fbw_all.transpose(2, 1, 0))  # [C, NS, B]

    in_maps = [
        {
            "ffw": np.ascontiguousarray(featsT[:, :, c * BL : (c + 1) * BL]),
            "fbw": np.ascontiguousarray(fbwT[:, :, c * BL : (c + 1) * BL]),
            "konst": konst,
        }
        for c in range(N_CORES)
    ]

    nc = _get_nc()
    res = run_bass_kernel_spmd(nc, in_maps, core_ids=list(range(N_CORES)))
    LAST_RESULT = res

    # ---- unshard / host assembly ----
    # no-overlap meeting:  logZ_b = log(Pf_t^T E Xb_m) + (L+1) s,
    # t = min(L, NS), m = L - t;  Xb_0 is the STOP one-hot, so m == 0 just
    # picks column C-1 of E.
    E32 = emat.astype(np.float32)
    logZ = np.zeros(B, dtype=np.float64)
    for c in range(N_CORES):
        h = np.asarray(res.results[c]["hist"]).reshape(C, HSLOTS, BL)
        for b in range(BL):
            bg = c * BL + b
            L = int(lengths[bg])
            t_b = min(L, NS)
            m_b = L - t_b
            P = h[:, _col_f(t_b), b].astype(np.float32)
            if m_b == 0:
                ex = E32[:, C - 1]
            else:
                X = h[:, _col_b(m_b), b].astype(np.float32)
                ex = E32 @ X
            logZ[bg] = np.log(float(P @ ex)) + (L + 1) * s
    fwd = np.float32(logZ.astype(np.float32).sum())

    # ---- gold-path score (host; pure gather/sum) ----
    r = np.arange(B)
    pad_start = np.concatenate([np.full((B, 1), C - 2, tags.dtype), tags], axis=1)
    pad_stop = np.concatenate([tags, np.full((B, 1), C - 1, tags.dtype)], axis=1)
    pad_stop[r, lengths] = C - 1
    tvals = transitions[pad_start, pad_stop]  # [B,T+1]
    t_score = np.cumsum(tvals, axis=1)[r, lengths].sum(dtype=np.float32)
    fg = np.take_along_axis(feats, tags[:, :, None], axis=2)[..., 0]
    f_score = np.where(mask.astype(bool), fg, np.float32(0.0)).sum(dtype=np.float32)

    nll = (np.float32(fwd) - (t_score + f_score)) / np.float32(B)
    return np.array(nll, dtype=np.float32)


# revision 6
# speedup vs baseline: 1.0052x; 1.0052x over previous
"""
CRFTagger NLL loss on 8 Trainium2 NeuronCores (Bass/Tile).

Strategy
--------
Data-parallel over batch: each of the 8 cores runs the CRF forward algorithm
for 16 of the 128 sequences.  The log-semiring scan runs in the *exp domain*
with a constant per-step shift s (s = log Perron-eigenvalue of exp(trans)
+ 0.5, computed on host from the transitions input), so one scan step is
just one PE matmul + one DVE elementwise multiply:

    P_{t+1} = (E^T @ P_t) * exp(feat_t)        E = exp(trans - s)  [C,C]

No per-step logsumexp / max / renormalization: with the Perron shift the
magnitudes drift only a few e-folds over the whole scan (validated:
|log P| < 10); states/weights are bf16 (fp32 exponent range, overflow-proof).

The scan is a latency-bound serial chain (~435ns per matmul+mul round trip),
so the kernel halves the chain length with a *bidirectional* split: a forward
recursion over the first half of time and an independent backward recursion
over the second half run concurrently, interleaved on the PE and DVE engines.
For variable-length sequences the backward pass is time-ALIGNED on host: the
feature stream of sequence b is reversed and shifted so every sequence "ends"
at the same device iteration, making the backward init a single shared
one-hot STOP seed.  Both chains run exactly NS=256 steps and meet with NO
feature overlap; the host closes the gap with one tiny E-matvec per
sequence:

    logZ_b = log( Pf_t^T  E  Xb_m ) + (len_b + 1) * s ,
    t = min(len_b, 256),  m = len_b - t

Both chains store their full state history in SBUF and ship it out
(overlapped with compute).  The history is laid out so the *tail* slots of
both chains (248..256) sit in one contiguous column range, letting the final
unoverlapped transfer be a single small DMA.  Regular ship points are chosen
off the feature-chunk boundaries (and staggered F vs B) so the ship DMAs
never collide with the chunk handoff on the serial chain.

The gold-path score (pure gathers over tags, O(B*T) with zero reuse) is
evaluated on host during unsharding.
"""

import sys

import ml_dtypes
import numpy as np

sys.path.insert(0, "/opt/trn_rl_repo")

import concourse.bacc as bacc  # noqa: E402
import concourse.mybir as mybir  # noqa: E402
from concourse import tile  # noqa: E402
from concourse.bass_utils import run_bass_kernel_spmd  # noqa: E402
from concourse.tile_rust import add_dep_helper  # noqa: E402

B, T, C = 128, 512, 128
N_CORES = 8
BL = B // N_CORES   # 16 sequences per core
NS = T // 2         # 256 steps per chain (both directions)

# state-history slot -> column layout: slots 0..247 of each chain live in
# two contiguous "body" regions, slots 248..NS (the tail, still in flight
# when the scan ends) of BOTH chains share one contiguous range so the
# final unoverlapped ship is a single DMA.
TAIL0 = 248                     # first tail slot
NTAIL = NS - TAIL0 + 1          # 9 tail slots per chain
HSLOTS = 2 * TAIL0 + 2 * NTAIL  # 514 total slots


def _col_f(s):  # forward-chain slot -> column group
    return s if s < TAIL0 else 2 * TAIL0 + (s - TAIL0)


def _col_b(s):  # backward-chain slot -> column group
    return TAIL0 + s if s < TAIL0 else 2 * TAIL0 + NTAIL + (s - TAIL0)


# feature chunk spans (steps); the first 16 steps of BOTH chains ride in one
# combined tensor/DMA ("fab0") so the scan starts after a single small
# transfer; later chunks are few and large (each chunk handoff costs the DVE
# a ~100ns first-touch wait), with switch points staggered F vs B
FW_SPANS = [(0, 16), (16, 144), (144, 256)]
BW_SPANS = [(0, 16), (16, 96), (96, 224), (224, 256)]
# ship points (fire after the TT that writes slot k+1 == point); all <= 247
# so regular ships stay inside the contiguous body regions, and none equals
# a chunk-switch step of either chain
FW_SHIPS = (64, 128, 192, 247)
BW_SHIPS = (32, 96, 160, 224, 247)

_NC = None
LAST_RESULT = None  # BassKernelResults of the most recent run (for profiling)


def _build_nc():
    nc = bacc.Bacc("TRN2", target_bir_lowering=False, debug=False)
    fp32 = mybir.dt.float32
    fp16 = mybir.dt.bfloat16
    fab0_h = nc.dram_tensor("fab0", [C, 32, BL], fp32, kind="ExternalInput")
    ffw_h = nc.dram_tensor("ffw", [C, NS, BL], fp32, kind="ExternalInput")
    fbw_h = nc.dram_tensor("fbw", [C, NS, BL], fp32, kind="ExternalInput")
    # one constant block = one DMA: [E | E^T | seedF | seedB]
    konst_h = nc.dram_tensor(
        "konst", [C, 2 * C + 2 * BL], fp16, kind="ExternalInput"
    )
    hist_h = nc.dram_tensor("hist", [C, HSLOTS * BL], fp16, kind="ExternalOutput")

    with tile.TileContext(nc) as tc:
        with (
            tc.tile_pool(name="consts", bufs=1) as consts,
            tc.tile_pool(name="ffw", bufs=len(FW_SPANS)) as ffwp,
            tc.tile_pool(name="fbw", bufs=len(BW_SPANS)) as fbwp,
            tc.tile_pool(name="hist", bufs=1) as hist,
            tc.tile_pool(name="mpsF", bufs=2, space="PSUM") as mpsF,
            tc.tile_pool(name="mpsB", bufs=2, space="PSUM") as mpsB,
        ):
            konst = consts.tile([C, 2 * C + 2 * BL], fp16)
            nc.sync.dma_start(out=konst[:], in_=konst_h[:])
            emat = konst[:, 0:C]
            ematT = konst[:, C : 2 * C]
            seedF = konst[:, 2 * C : 2 * C + BL]
            seedB = konst[:, 2 * C + BL : 2 * C + 2 * BL]

            # both chains' state histories in one tile (slot->column maps
            # above); slot 0 (the seed) lives in the konst tile instead —
            # the host never reads slot 0 of the shipped history.
            HIST = hist.tile([C, HSLOTS * BL], fp16)

            # stream feats in chunks, exponentiating in place
            def load_feats(pool, dram, lo, hi):
                f = pool.tile([C, (hi - lo) * BL], fp32)
                nc.sync.dma_start(
                    out=f[:],
                    in_=dram[:, lo:hi, :].rearrange("c t b -> c (t b)"),
                )
                nc.scalar.activation(
                    f[:], f[:], mybir.ActivationFunctionType.Exp
                )
                return f

            def slot_map(spans):
                m = {}
                for i, (lo, hi) in enumerate(spans):
                    for k in range(lo, hi):
                        m[k] = (i, k - lo)
                return m

            # both chains' first 16 steps arrive in ONE DMA right behind
            # konst; the F half is exp'd first (its TT fires first), then the
            # B half.  Remaining chunks stream in interleaved F/B so neither
            # chain's next chunk queues behind all of the other chain's DMAs.
            fab0 = ffwp.tile([C, 32 * BL], fp32)
            nc.sync.dma_start(
                out=fab0[:], in_=fab0_h[:].rearrange("c t b -> c (t b)")
            )
            for half in range(2):
                sl = fab0[:, half * 16 * BL : (half + 1) * 16 * BL]
                nc.scalar.activation(
                    sl, sl, mybir.ActivationFunctionType.Exp
                )
            ffw, fbw = [fab0], [fab0]
            for i in range(1, max(len(FW_SPANS), len(BW_SPANS))):
                if i < len(FW_SPANS):
                    ffw.append(load_feats(ffwp, ffw_h, *FW_SPANS[i]))
                if i < len(BW_SPANS):
                    fbw.append(load_feats(fbwp, fbw_h, *BW_SPANS[i]))
            # slot maps give (tile_idx, offset); B's offsets in the shared
            # fab0 tile sit after F's 16 slots
            fw_slot = slot_map(FW_SPANS)
            bw_slot = {k: (i, j + (16 if i == 0 else 0))
                       for k, (i, j) in slot_map(BW_SPANS).items()}

            def step(k, psum_pool, wmat, colmap, ftiles, fslot, seed,
                     ships, shipped, phase_after=None):
                m = psum_pool.tile([C, BL], mybir.dt.float32)
                if k == 0:
                    rhs = seed
                else:
                    c0 = colmap(k) * BL
                    rhs = HIST[:, c0 : c0 + BL]
                mm = nc.tensor.matmul(
                    m[:], wmat, rhs, start=True, stop=True,
                )
                if phase_after is not None:
                    # pure scheduling edge: pins this chain's phase a fixed
                    # lag behind the other chain so the two never collapse
                    # into the in-phase (serialized, 2x slower) mode
                    add_dep_helper(
                        mm.ins, phase_after.ins, sync=True,
                        reason="cross-chain phase pin",
                    )
                i, j = fslot[k]
                f = ftiles[i]
                co = colmap(k + 1) * BL
                tt = nc.vector.tensor_mul(
                    HIST[:, co : co + BL],
                    f[:, j * BL : (j + 1) * BL],
                    m[:],
                )
                # ship finished body slots while the scan keeps running
                if k + 1 in ships:
                    lo = colmap(shipped["s"]) * BL
                    hi = (colmap(k + 1) + 1) * BL
                    nc.sync.dma_start(
                        out=hist_h[:, lo:hi], in_=HIST[:, lo:hi]
                    )
                    shipped["s"] = k + 2
                return tt

            # Stagger the backward chain one step behind the forward chain in
            # each engine's (in-order) instruction stream, and pin its phase
            # with an explicit cross-chain edge, so B's ops always slot into
            # the idle gaps of F's latency-bound period instead of collapsing
            # into the in-phase (serialized, 2x slower) mode.
            prev_ttF = None
            shipF, shipB = {"s": 1}, {"s": 1}
            for k in range(NS):
                ttF = step(k, mpsF, emat, _col_f, ffw, fw_slot, seedF,
                           FW_SHIPS, shipF)
                if k >= 1:
                    step(k - 1, mpsB, ematT, _col_b, fbw, bw_slot, seedB,
                         BW_SHIPS, shipB, phase_after=prev_ttF)
                prev_ttF = ttF
            step(NS - 1, mpsB, ematT, _col_b, fbw, bw_slot, seedB,
                 BW_SHIPS, shipB, phase_after=prev_ttF)

            # single merged tail ship: slots 248..256 of BOTH chains are one
            # contiguous column range by construction
            lo = 2 * TAIL0 * BL
            nc.sync.dma_start(
                out=hist_h[:, lo:], in_=HIST[:, lo:]
            )
    nc.compile()
    return nc


def _get_nc():
    global _NC
    if _NC is None:
        _NC = _build_nc()
    return _NC


def _shift_constant(transitions: np.ndarray) -> float:
    """log(Perron eigenvalue of exp(trans)) + E[e^feat] growth correction."""
    tm = transitions.astype(np.float64)
    mx = tm.max()
    Et = np.exp(tm - mx)
    v = np.ones(C) / C
    r = 1.0
    for _ in range(200):
        w = Et.T @ v
        r = np.linalg.norm(w)
        v = w / r
    return float(np.log(r) + mx + 0.5)


def kernel(feats, mask, tags, transitions):
    global LAST_RESULT
    feats = np.asarray(feats, dtype=np.float32)
    mask = np.asarray(mask, dtype=np.int32)
    tags = np.asarray(tags, dtype=np.int32)
    transitions = np.asarray(transitions, dtype=np.float32)

    s = _shift_constant(transitions)
    with np.errstate(under="ignore"):
        emat = np.exp(
            (transitions.astype(np.float64) - s).astype(np.float32)
        ).astype(ml_dtypes.bfloat16)

    konst = np.zeros((C, 2 * C + 2 * BL), dtype=ml_dtypes.bfloat16)
    konst[:, :C] = emat
    konst[:, C : 2 * C] = emat.T
    konst[C - 2, 2 * C : 2 * C + BL] = 1.0        # forward seed: START one-hot
    konst[C - 1, 2 * C + BL : 2 * C + 2 * BL] = 1.0  # backward seed: STOP

    lengths = mask.sum(1)  # [B]

    # forward feats: [B,T,C] -> [C, NS, B] slices per core
    featsT = np.ascontiguousarray(feats[:, :NS, :].transpose(2, 1, 0))
    # backward aligned feats: iteration m of sequence b consumes
    # feats[b, len_b-1-m, :]; entries past the sequence start are 0 (exp -> 1)
    fbw_all = np.zeros((B, NS, C), dtype=np.float32)
    for b in range(B):
        L = int(lengths[b])
        n = min(L, NS)
        fbw_all[b, :n] = feats[b, L - n : L][::-1]
    fbwT = np.ascontiguousarray(fbw_all.transpose(2, 1, 0))  # [C, NS, B]

    in_maps = [
        {
            "fab0": np.ascontiguousarray(
                np.concatenate(
                    [
                        featsT[:, :16, c * BL : (c + 1) * BL],
                        fbwT[:, :16, c * BL : (c + 1) * BL],
                    ],
                    axis=1,
                )
            ),
            "ffw": np.ascontiguousarray(featsT[:, :, c * BL : (c + 1) * BL]),
            "fbw": np.ascontiguousarray(fbwT[:, :, c * BL : (c + 1) * BL]),
            "konst": konst,
        }
        for c in range(N_CORES)
    ]

    nc = _get_nc()
    res = run_bass_kernel_spmd(nc, in_maps, core_ids=list(range(N_CORES)))
    LAST_RESULT = res

    # ---- unshard / host assembly ----
    # no-overlap meeting:  logZ_b = log(Pf_t^T E Xb_m) + (L+1) s,
    # t = min(L, NS), m = L - t;  Xb_0 is the STOP one-hot, so m == 0 just
    # picks column C-1 of E.
    E32 = emat.astype(np.float32)
    logZ = np.zeros(B, dtype=np.float64)
    for c in range(N_CORES):
        h = np.asarray(res.results[c]["hist"]).reshape(C, HSLOTS, BL)
        for b in range(BL):
            bg = c * BL + b
            L = int(lengths[bg])
            t_b = min(L, NS)
            m_b = L - t_b
            P = h[:, _col_f(t_b), b].astype(np.float32)
            if m_b == 0:
                ex = E32[:, C - 1]
            else:
                X = h[:, _col_b(m_b), b].astype(np.float32)
                ex = E32 @ X
            logZ[bg] = np.log(float(P @ ex)) + (L + 1) * s
    fwd = np.float32(logZ.astype(np.float32).sum())

    # ---- gold-path score (host; pure gather/sum) ----
    r = np.arange(B)
    pad_start = np.concatenate([np.full((B, 1), C - 2, tags.dtype), tags], axis=1)
    pad_stop = np.concatenate([tags, np.full((B, 1), C - 1, tags.dtype)], axis=1)
    pad_stop[r, lengths] = C - 1
    tvals = transitions[pad_start, pad_stop]  # [B,T+1]
    t_score = np.cumsum(tvals, axis=1)[r, lengths].sum(dtype=np.float32)
    fg = np.take_along_axis(feats, tags[:, :, None], axis=2)[..., 0]
    f_score = np.where(mask.astype(bool), fg, np.float32(0.0)).sum(dtype=np.float32)

    nll = (np.float32(fwd) - (t_score + f_score)) / np.float32(B)
    return np.array(nll, dtype=np.float32)


# revision 12
# speedup vs baseline: 1.0168x; 1.0115x over previous
"""
CRFTagger NLL loss on 8 Trainium2 NeuronCores (Bass/Tile).

Strategy
--------
Data-parallel over batch: each of the 8 cores runs the CRF forward algorithm
for 16 of the 128 sequences.  The log-semiring scan runs in the *exp domain*
with a constant per-step shift s (s = log Perron-eigenvalue of exp(trans)
+ 0.5, computed on host from the transitions input), so one scan step is
just one PE matmul + one DVE elementwise multiply:

    P_{t+1} = (E^T @ P_t) * exp(feat_t)        E = exp(trans - s)  [C,C]

No per-step logsumexp / max / renormalization: with the Perron shift the
magnitudes drift only a few e-folds over the whole scan (validated:
|log P| < 10); states/weights are bf16 (fp32 exponent range, overflow-proof).

The scan is a latency-bound serial chain (~435ns per matmul+mul round trip),
so the kernel halves the chain length with a *bidirectional* split: a forward
recursion over the first half of time and an independent backward recursion
over the second half run concurrently, interleaved on the PE and DVE engines.
For variable-length sequences the backward pass is time-ALIGNED on host: the
feature stream of sequence b is reversed and shifted so every sequence "ends"
at the same device iteration, making the backward init a single shared
one-hot STOP seed.  Both chains run exactly NS=256 steps and meet with NO
feature overlap; the host closes the gap with one tiny E-matvec per
sequence:

    logZ_b = log( Pf_t^T  E  Xb_m ) + (len_b + 1) * s ,
    t = min(len_b, 256),  m = len_b - t

Both chains store their full state history in SBUF and ship it out
(overlapped with compute).  The history is laid out so the *tail* slots of
both chains (248..256) sit in one contiguous column range, letting the final
unoverlapped transfer be a single small DMA.  Regular ship points are chosen
off the feature-chunk boundaries (and staggered F vs B) so the ship DMAs
never collide with the chunk handoff on the serial chain.

The gold-path score (pure gathers over tags, O(B*T) with zero reuse) is
evaluated on host during unsharding.
"""

import sys

import ml_dtypes
import numpy as np

sys.path.insert(0, "/opt/trn_rl_repo")

import concourse.bacc as bacc  # noqa: E402
import concourse.mybir as mybir  # noqa: E402
from concourse import tile  # noqa: E402
from concourse.bass_utils import run_bass_kernel_spmd  # noqa: E402
from concourse.tile_rust import add_dep_helper  # noqa: E402

B, T, C = 128, 512, 128
N_CORES = 8
BL = B // N_CORES   # 16 sequences per core
NS = T // 2         # 256 steps per chain (both directions)

# state-history slot -> column layout: slots 0..247 of each chain live in
# two contiguous "body" regions, slots 248..NS (the tail, still in flight
# when the scan ends) of BOTH chains share one contiguous range so the
# final unoverlapped ship is a single DMA.
TAIL0 = 248                     # first tail slot
NTAIL = NS - TAIL0 + 1          # 9 tail slots per chain
HSLOTS = 2 * TAIL0 + 2 * NTAIL  # 514 total slots


def _col_f(s):  # forward-chain slot -> column group
    return s if s < TAIL0 else 2 * TAIL0 + (s - TAIL0)


def _col_b(s):  # backward-chain slot -> column group
    return TAIL0 + s if s < TAIL0 else 2 * TAIL0 + NTAIL + (s - TAIL0)


# feature chunk spans (steps); the first 16 steps of BOTH chains ride in one
# combined tensor/DMA ("fab0") so the scan starts after a single small
# transfer; later chunks are few and large (each chunk handoff costs the DVE
# a ~100ns first-touch wait), with switch points staggered F vs B
FW_SPANS = [(0, 16), (16, 144), (144, 256)]
BW_SPANS = [(0, 16), (16, 96), (96, 224), (224, 256)]
# ship points (fire after the TT that writes slot k+1 == point); all <= 247
# so regular ships stay inside the contiguous body regions, and none equals
# a chunk-switch step of either chain
FW_SHIPS = (64, 128, 192, 247)
BW_SHIPS = (32, 96, 160, 224, 247)

_NC = None
LAST_RESULT = None  # BassKernelResults of the most recent run (for profiling)


def _build_nc():
    nc = bacc.Bacc("TRN2", target_bir_lowering=False, debug=False)
    fp32 = mybir.dt.float32
    fp16 = mybir.dt.bfloat16
    ffw_h = nc.dram_tensor("ffw", [C, NS, BL], fp16, kind="ExternalInput")
    fbw_h = nc.dram_tensor("fbw", [C, NS, BL], fp16, kind="ExternalInput")
    # one constant block = one DMA: [E | E^T | seedF | seedB | F feats 0..15
    # | B feats 0..15] — riding the first 16 steps of both chains in the
    # same transfer means a single DMA flight gates the whole scan start
    KBASE = 2 * C + 2 * BL
    konst_h = nc.dram_tensor(
        "konst", [C, KBASE + 2 * 16 * BL], fp16, kind="ExternalInput"
    )
    hist_h = nc.dram_tensor("hist", [C, HSLOTS * BL], fp16, kind="ExternalOutput")

    with tile.TileContext(nc) as tc:
        with (
            tc.tile_pool(name="consts", bufs=1) as consts,
            tc.tile_pool(name="ffw", bufs=len(FW_SPANS)) as ffwp,
            tc.tile_pool(name="fbw", bufs=len(BW_SPANS)) as fbwp,
            tc.tile_pool(name="hist", bufs=1) as hist,
            tc.tile_pool(name="mpsF", bufs=2, space="PSUM") as mpsF,
            tc.tile_pool(name="mpsB", bufs=2, space="PSUM") as mpsB,
        ):
            konst = consts.tile([C, KBASE + 2 * 16 * BL], fp16)
            nc.sync.dma_start(out=konst[:], in_=konst_h[:])
            emat = konst[:, 0:C]
            ematT = konst[:, C : 2 * C]
            seedF = konst[:, 2 * C : 2 * C + BL]
            seedB = konst[:, 2 * C + BL : 2 * C + 2 * BL]

            # both chains' state histories in one tile (slot->column maps
            # above); slot 0 (the seed) lives in the konst tile instead —
            # the host never reads slot 0 of the shipped history.
            HIST = hist.tile([C, HSLOTS * BL], fp16)

            # stream feats in chunks (bf16), exponentiating in place
            def load_feats(pool, dram, lo, hi):
                f = pool.tile([C, (hi - lo) * BL], fp16)
                nc.sync.dma_start(
                    out=f[:],
                    in_=dram[:, lo:hi, :].rearrange("c t b -> c (t b)"),
                )
                nc.scalar.activation(
                    f[:], f[:], mybir.ActivationFunctionType.Exp
                )
                return f

            def slot_map(spans):
                m = {}
                for i, (lo, hi) in enumerate(spans):
                    for k in range(lo, hi):
                        m[k] = (i, k - lo)
                return m

            # both chains' first 16 steps live in the konst tile; exp the F
            # half first (its TT fires first), then the B half.  Remaining
            # chunks stream in interleaved F/B so neither chain's next chunk
            # queues behind all of the other chain's DMAs.
            for half in range(2):
                sl = konst[:, KBASE + half * 16 * BL : KBASE + (half + 1) * 16 * BL]
                nc.scalar.activation(
                    sl, sl, mybir.ActivationFunctionType.Exp
                )
            # feature-tile list entries are (tile, base column)
            ffw = [(konst, KBASE)]
            fbw = [(konst, KBASE + 16 * BL)]
            for i in range(1, max(len(FW_SPANS), len(BW_SPANS))):
                if i < len(FW_SPANS):
                    ffw.append((load_feats(ffwp, ffw_h, *FW_SPANS[i]), 0))
                if i < len(BW_SPANS):
                    fbw.append((load_feats(fbwp, fbw_h, *BW_SPANS[i]), 0))
            fw_slot = slot_map(FW_SPANS)
            bw_slot = slot_map(BW_SPANS)

            def step(k, psum_pool, wmat, colmap, ftiles, fslot, seed,
                     ships, shipped, phase_after=None):
                m = psum_pool.tile([C, BL], mybir.dt.float32)
                if k == 0:
                    rhs = seed
                else:
                    c0 = colmap(k) * BL
                    rhs = HIST[:, c0 : c0 + BL]
                mm = nc.tensor.matmul(
                    m[:], wmat, rhs, start=True, stop=True,
                )
                if phase_after is not None:
                    # pure scheduling edge: pins this chain's phase a fixed
                    # lag behind the other chain so the two never collapse
                    # into the in-phase (serialized, 2x slower) mode
                    add_dep_helper(
                        mm.ins, phase_after.ins, sync=True,
                        reason="cross-chain phase pin",
                    )
                i, j = fslot[k]
                f, fbase = ftiles[i]
                co = colmap(k + 1) * BL
                tt = nc.vector.tensor_mul(
                    HIST[:, co : co + BL],
                    f[:, fbase + j * BL : fbase + (j + 1) * BL],
                    m[:],
                )
                # ship finished body slots while the scan keeps running
                if k + 1 in ships:
                    lo = colmap(shipped["s"]) * BL
                    hi = (colmap(k + 1) + 1) * BL
                    nc.sync.dma_start(
                        out=hist_h[:, lo:hi], in_=HIST[:, lo:hi]
                    )
                    shipped["s"] = k + 2
                return tt

            # Stagger the backward chain one step behind the forward chain in
            # each engine's (in-order) instruction stream, and pin its phase
            # with an explicit cross-chain edge, so B's ops always slot into
            # the idle gaps of F's latency-bound period instead of collapsing
            # into the in-phase (serialized, 2x slower) mode.
            prev_ttF = None
            shipF, shipB = {"s": 1}, {"s": 1}
            for k in range(NS):
                ttF = step(k, mpsF, emat, _col_f, ffw, fw_slot, seedF,
                           FW_SHIPS, shipF)
                if k >= 1:
                    step(k - 1, mpsB, ematT, _col_b, fbw, bw_slot, seedB,
                         BW_SHIPS, shipB, phase_after=prev_ttF)
                prev_ttF = ttF
            step(NS - 1, mpsB, ematT, _col_b, fbw, bw_slot, seedB,
                 BW_SHIPS, shipB, phase_after=prev_ttF)

            # single merged tail ship: slots 248..256 of BOTH chains are one
            # contiguous column range by construction
            lo = 2 * TAIL0 * BL
            nc.sync.dma_start(
                out=hist_h[:, lo:], in_=HIST[:, lo:]
            )
    nc.compile()
    return nc


def _get_nc():
    global _NC
    if _NC is None:
        _NC = _build_nc()
    return _NC


def _shift_constant(transitions: np.ndarray) -> float:
    """log(Perron eigenvalue of exp(trans)) + E[e^feat] growth correction."""
    tm = transitions.astype(np.float64)
    mx = tm.max()
    Et = np.exp(tm - mx)
    v = np.ones(C) / C
    r = 1.0
    for _ in range(200):
        w = Et.T @ v
        r = np.linalg.norm(w)
        v = w / r
    return float(np.log(r) + mx + 0.5)


def kernel(feats, mask, tags, transitions):
    global LAST_RESULT
    feats = np.asarray(feats, dtype=np.float32)
    mask = np.asarray(mask, dtype=np.int32)
    tags = np.asarray(tags, dtype=np.int32)
    transitions = np.asarray(transitions, dtype=np.float32)

    s = _shift_constant(transitions)
    with np.errstate(under="ignore"):
        emat = np.exp(
            (transitions.astype(np.float64) - s).astype(np.float32)
        ).astype(ml_dtypes.bfloat16)

    KBASE = 2 * C + 2 * BL
    konst0 = np.zeros((C, KBASE + 2 * 16 * BL), dtype=ml_dtypes.bfloat16)
    konst0[:, :C] = emat
    konst0[:, C : 2 * C] = emat.T
    konst0[C - 2, 2 * C : 2 * C + BL] = 1.0        # forward seed: START one-hot
    konst0[C - 1, 2 * C + BL : 2 * C + 2 * BL] = 1.0  # backward seed: STOP

    lengths = mask.sum(1)  # [B]

    # forward feats: [B,T,C] -> [C, NS, B] slices per core (bf16 on the wire)
    featsT = np.ascontiguousarray(
        feats[:, :NS, :].transpose(2, 1, 0).astype(ml_dtypes.bfloat16)
    )
    # backward aligned feats: iteration m of sequence b consumes
    # feats[b, len_b-1-m, :]; entries past the sequence start are 0 (exp -> 1)
    fbw_all = np.zeros((B, NS, C), dtype=np.float32)
    for b in range(B):
        L = int(lengths[b])
        n = min(L, NS)
        fbw_all[b, :n] = feats[b, L - n : L][::-1]
    fbwT = np.ascontiguousarray(
        fbw_all.transpose(2, 1, 0).astype(ml_dtypes.bfloat16)
    )  # [C, NS, B]

    in_maps = []
    for c in range(N_CORES):
        sl = slice(c * BL, (c + 1) * BL)
        konst = konst0.copy()
        konst[:, KBASE : KBASE + 16 * BL] = featsT[:, :16, sl].reshape(C, -1)
        konst[:, KBASE + 16 * BL :] = fbwT[:, :16, sl].reshape(C, -1)
        in_maps.append(
            {
                "ffw": np.ascontiguousarray(featsT[:, :, sl]),
                "fbw": np.ascontiguousarray(fbwT[:, :, sl]),
                "konst": konst,
            }
        )

    nc = _get_nc()
    res = run_bass_kernel_spmd(nc, in_maps, core_ids=list(range(N_CORES)))
    LAST_RESULT = res

    # ---- unshard / host assembly ----
    # no-overlap meeting:  logZ_b = log(Pf_t^T E Xb_m) + (L+1) s,
    # t = min(L, NS), m = L - t;  Xb_0 is the STOP one-hot, so m == 0 just
    # picks column C-1 of E.
    E32 = emat.astype(np.float32)
    logZ = np.zeros(B, dtype=np.float64)
    for c in range(N_CORES):
        h = np.asarray(res.results[c]["hist"]).reshape(C, HSLOTS, BL)
        for b in range(BL):
            bg = c * BL + b
            L = int(lengths[bg])
            t_b = min(L, NS)
            m_b = L - t_b
            P = h[:, _col_f(t_b), b].astype(np.float32)
            if m_b == 0:
                ex = E32[:, C - 1]
            else:
                X = h[:, _col_b(m_b), b].astype(np.float32)
                ex = E32 @ X
            logZ[bg] = np.log(float(P @ ex)) + (L + 1) * s
    fwd = np.float32(logZ.astype(np.float32).sum())

    # ---- gold-path score (host; pure gather/sum) ----
    r = np.arange(B)
    pad_start = np.concatenate([np.full((B, 1), C - 2, tags.dtype), tags], axis=1)
    pad_stop = np.concatenate([tags, np.full((B, 1), C - 1, tags.dtype)], axis=1)
    pad_stop[r, lengths] = C - 1
    tvals = transitions[pad_start, pad_stop]  # [B,T+1]
    t_score = np.cumsum(tvals, axis=1)[r, lengths].sum(dtype=np.float32)
    fg = np.take_along_axis(feats, tags[:, :, None], axis=2)[..., 0]
    f_score = np.where(mask.astype(bool), fg, np.float32(0.0)).sum(dtype=np.float32)

    nll = (np.float32(fwd) - (t_score + f_score)) / np.float32(B)
    return np.array(nll, dtype=np.float32)
